# revision 31
# baseline (speedup 1.0000x reference)
"""Trainium2 Bass kernel for ConvexLORALinear: out = (input @ lora_A) @ lora_B.

Full shapes: input [8192, 4096] f32, lora_A [4096, 128] f32, lora_B [128, 4096] f32.
Sharding: data-parallel on the token dim — each of the 8 cores gets 1024 tokens,
lora_A / lora_B replicated. No collectives.

Per-core dataflow (build_nc4, the shipped config — all DMAs natural/contiguous):
  1. input rows arrive as [128t, 4096k] f32 tiles and are cast to bf16 on the
     Pool engine; the contraction dim (k) must sit on SBUF partitions for the
     PE, so each [128,128] bf16 block is transposed on the PE (transpose-mode
     matmul against a bf16 identity, 1 cycle/row) and evicted to a bf16 itp
     tile by DVE/ACT.
  2. mm1: C1T[r, t512] += A[kc].T @ inputT[kc, t512] accumulated over kc in
     PSUM (f32), operands bf16 (512-wide moving free dim streams at full
     rate on hardware; 256 measured slower).
  3. mm2: out[t128, n512] = C1T[:, t128].T @ B[:, n512], operands bf16,
     evicted f32 and stored as full [128, 4096] rows on the ACT DMA ring.
bf16 rounding of input/A/B/C1 lands at ~1.9e-3 scale-relative error vs the
2e-2 gate. Older builders (build_nc/2/3: f32/f32r pipeline, 2-cycle/row f32
transposes) are kept for comparison runs.
"""

import os
import sys

import numpy as np

try:
    import concourse.bass as bass  # noqa: F401
except ImportError:  # concourse not on sys.path in this interpreter
    for _p in ("/opt/trn_rl_repo", os.path.expanduser("~/trn_rl_repo")):
        if os.path.isdir(_p) and _p not in sys.path:
            sys.path.insert(0, _p)
    import concourse.bass as bass

import concourse.mybir as mybir
from concourse.bass_utils import run_bass_kernel_spmd
from concourse.masks import make_identity
from concourse.tile import TileContext

P = 128
FREE = 512  # matmul moving-operand free dim (f32 PSUM bank = 512 floats)

N_CORES = 8
T_FULL = 8192
D_IN = 4096
RANK = 128
D_OUT = 4096

F32 = mybir.dt.float32


def _legalize_waits(nc: bass.Bass, cap: int = 1) -> None:
    """Split instructions carrying >cap semaphore waits.

    The walrus build in this environment rejects instructions with several
    sync-wait commands (seen on the TileContext tail drain: "Too many sync
    wait commands").  Hoist excess waits onto same-engine NOPs placed
    immediately before the instruction — the engine stream is serial, so
    waiting earlier on the same engine is equivalent.
    """
    n = 0
    for fn in nc.m.functions:
        for bb in fn.blocks:
            insts = bb.instructions
            new_list = []
            for inst in insts:
                si = inst.sync_info
                if si is not None and si.on_wait and len(si.on_wait) > cap:
                    waits = list(si.on_wait)
                    for w in waits[:-cap]:
                        nop = mybir.InstNoOp(
                            name=f"waitsplit-{inst.name}-{n}", ins=[], outs=[]
                        )
                        n += 1
                        nop.engine = inst.engine
                        nop.sync_info = mybir.SyncInfo(on_wait=[w], on_update=[])
                        new_list.append(nop)
                    inst.sync_info = mybir.SyncInfo(
                        on_wait=waits[-cap:], on_update=list(si.on_update or [])
                    )
                new_list.append(inst)
            insts[:] = new_list


def build_nc(
    t_core: int = T_FULL // N_CORES,
    d_in: int = D_IN,
    rank: int = RANK,
    d_out: int = D_OUT,
    mm_dt: mybir.dt = mybir.dt.float32r,
    legalize: bool = True,
    passes: int = 1,  # re-run the body N times inside one NEFF (timing aid)
) -> bass.Bass:
    assert t_core % FREE == 0 and d_in % P == 0 and d_out % FREE == 0
    assert rank == P, "kernel assumes rank == 128 (single contraction tile in mm2)"
    n_t_tiles = t_core // FREE  # 512-token slabs
    n_j = FREE // P  # 128-token blocks per slab
    n_kc = d_in // P  # contraction chunks for mm1
    n_nc = d_out // FREE  # output column chunks
    out_cols = min(d_out, 2048)  # SBUF output staging width per DMA
    n_halves = d_out // out_cols

    nc = bass.Bass()
    inp = nc.declare_dram_parameter("input", [t_core, d_in], F32, isOutput=False)
    a = nc.declare_dram_parameter("lora_A", [d_in, rank], F32, isOutput=False)
    b = nc.declare_dram_parameter("lora_B", [rank, d_out], F32, isOutput=False)
    outp = nc.declare_dram_parameter("output", [t_core, d_out], F32, isOutput=True)

    with TileContext(nc) as tc:
        with (
            tc.tile_pool(name="const", bufs=1) as const_pool,
            tc.tile_pool(name="a_sb", bufs=1) as a_pool,
            tc.tile_pool(name="b_sb", bufs=1) as b_pool,
            tc.tile_pool(name="nat", bufs=3) as nat_pool,
            tc.tile_pool(name="itp", bufs=n_kc + 2) as itp_pool,
            tc.tile_pool(name="c1t_sb", bufs=2) as c1t_pool,
            tc.tile_pool(name="out_sb", bufs=2) as out_pool,
            tc.tile_pool(name="tr_ps", bufs=4, space="PSUM") as tr_psum,
            tc.tile_pool(name="c1t_ps", bufs=2, space="PSUM") as c1t_psum,
            tc.tile_pool(name="out_ps", bufs=2, space="PSUM") as out_psum,
        ):
            identity = const_pool.tile([P, P], F32)
            make_identity(nc, identity)

            # A as [p, kc, r]: slice [:, kc, :] = A[kc*128:(kc+1)*128, :].
            # fp32r matmul operands must be produced pre-rounded to fp32r, so
            # DMA into an f32 staging tile and cast-copy into the fp32r tile.
            a_stage = a_pool.tile([P, n_kc, rank], F32, name="a_stage")
            nc.sync.dma_start(
                out=a_stage[:], in_=a.rearrange("(kc p) r -> p kc r", p=P)
            )
            a_sb = a_pool.tile([P, n_kc, rank], mm_dt, name="a_sb")
            nc.vector.tensor_copy(a_sb[:], a_stage[:])
            b_stage = b_pool.tile([P, d_out], F32, name="b_stage")
            nc.sync.dma_start(out=b_stage[:], in_=b[:, :])
            b_sb = b_pool.tile([P, d_out], mm_dt, name="b_sb")
            nc.scalar.copy(b_sb[:], b_stage[:])

            n_copy = 0  # alternation counter for DVE/ACT eviction balance

            def evict(dst, src):
                nonlocal n_copy
                if n_copy % 2 == 0:
                    nc.vector.tensor_copy(dst, src)
                else:
                    nc.scalar.copy(dst, src)
                n_copy += 1

            for pss in range(passes):
              for tt in range(n_t_tiles):
                itps = [
                    itp_pool.tile(
                        [P, FREE], mm_dt, tag="itp", name=f"itp{pss}_{tt}_{i}"
                    )
                    for i in range(n_kc)
                ]
                for j in range(n_j):
                    tb = tt * n_j + j
                    nat = nat_pool.tile([P, d_in], F32)
                    nc.sync.dma_start(out=nat[:], in_=inp[tb * P : (tb + 1) * P, :])
                    for kc in range(n_kc):
                        trp = tr_psum.tile([P, P], F32)
                        nc.tensor.matmul(
                            trp[:],
                            nat[:, kc * P : (kc + 1) * P],
                            identity[:],
                            is_transpose=True,
                            start=True,
                            stop=True,
                        )
                        evict(itps[kc][:, j * P : (j + 1) * P], trp[:])
                # mm1: C1T[r, t] accumulated over kc
                c1t_ps = c1t_psum.tile([P, FREE], F32)
                for kc in range(n_kc):
                    nc.tensor.matmul(
                        c1t_ps[:],
                        a_sb[:, kc, :],
                        itps[kc][:],
                        start=(kc == 0),
                        stop=(kc == n_kc - 1),
                    )
                c1t = c1t_pool.tile([P, FREE], mm_dt)
                nc.vector.tensor_copy(c1t[:, : FREE // 2], c1t_ps[:, : FREE // 2])
                nc.scalar.copy(c1t[:, FREE // 2 :], c1t_ps[:, FREE // 2 :])
                # mm2: out[t, n] = C1T[:, t].T @ B[:, n]
                for j in range(n_j):
                    tb = tt * n_j + j
                    for h in range(n_halves):
                        o_sb = out_pool.tile([P, out_cols], F32)
                        for q in range(n_nc // n_halves):
                            ncol = h * (n_nc // n_halves) + q
                            o_ps = out_psum.tile([P, FREE], F32)
                            nc.tensor.matmul(
                                o_ps[:],
                                c1t[:, j * P : (j + 1) * P],
                                b_sb[:, ncol * FREE : (ncol + 1) * FREE],
                                start=True,
                                stop=True,
                            )
                            evict(o_sb[:, q * FREE : (q + 1) * FREE], o_ps[:])
                        nc.sync.dma_start(
                            out=outp[
                                tb * P : (tb + 1) * P,
                                h * out_cols : (h + 1) * out_cols,
                            ],
                            in_=o_sb[:],
                        )
    if legalize:
        _legalize_waits(nc)
    return nc


def build_nc2(
    t_core: int = T_FULL // N_CORES,
    d_in: int = D_IN,
    rank: int = RANK,
    d_out: int = D_OUT,
    mm_dt: mybir.dt = mybir.dt.float32r,
    legalize: bool = True,
    passes: int = 1,
    skip_tr: bool = False,  # timing probe only: omit transposes (wrong results)
    skip_mm: bool = False,  # timing probe only: DMA round-trip kernel
    t_tile: int = 512,  # token-tile width (mm1 moving free dim, >=256)
    itp_bufs: int | None = None,
    ident_bf16: bool = False,  # bf16 identity for transpose-mode matmuls
    store_act: bool = False,  # issue output stores on the ACT HWDGE ring
) -> bass.Bass:
    """v2 layout: transposes grouped 4-per-PSUM-bank -> one [128,4,128]
    eviction each; inputT staged in one [P, n_kc, t_tile] tile; A/B staged
    through the recycled nat pool."""
    assert t_core % t_tile == 0 and d_in % P == 0 and d_out % FREE == 0
    assert rank == P and t_tile >= 256
    n_t_tiles = t_core // t_tile
    n_j = t_tile // P
    if itp_bufs is None:
        itp_bufs = 2 if t_tile <= 256 else 1
    n_kc = d_in // P
    n_nc = d_out // FREE
    out_cols = min(d_out, 2048)
    n_halves = d_out // out_cols
    QUAD = 4
    n_q = n_kc // QUAD

    nc = bass.Bass()
    inp = nc.declare_dram_parameter("input", [t_core, d_in], F32, isOutput=False)
    a = nc.declare_dram_parameter("lora_A", [d_in, rank], F32, isOutput=False)
    b = nc.declare_dram_parameter("lora_B", [rank, d_out], F32, isOutput=False)
    outp = nc.declare_dram_parameter("output", [t_core, d_out], F32, isOutput=True)

    with TileContext(nc) as tc:
        with (
            tc.tile_pool(name="const", bufs=1) as const_pool,
            tc.tile_pool(name="a_sb", bufs=1) as a_pool,
            tc.tile_pool(name="b_sb", bufs=1) as b_pool,
            tc.tile_pool(name="nat", bufs=3) as nat_pool,
            tc.tile_pool(name="itp", bufs=itp_bufs) as itp_pool,
            tc.tile_pool(name="c1t_sb", bufs=2) as c1t_pool,
            tc.tile_pool(name="out_sb", bufs=2) as out_pool,
            tc.tile_pool(name="tr_ps", bufs=4, space="PSUM") as tr_psum,
            tc.tile_pool(name="c1t_ps", bufs=2, space="PSUM") as c1t_psum,
            tc.tile_pool(name="out_ps", bufs=2, space="PSUM") as out_psum,
        ):
            identity = const_pool.tile([P, P], mybir.dt.bfloat16 if ident_bf16 else F32)
            make_identity(nc, identity)

            a_stage = nat_pool.tile([P, d_in], F32, tag="nat", name="a_stage")
            nc.sync.dma_start(
                out=a_stage[:].rearrange("p (kc r) -> p kc r", r=rank),
                in_=a.rearrange("(kc p) r -> p kc r", p=P),
            )
            a_sb = a_pool.tile([P, d_in], mm_dt)
            nc.vector.tensor_copy(a_sb[:], a_stage[:])
            b_stage = nat_pool.tile([P, d_out], F32, tag="nat", name="b_stage")
            nc.sync.dma_start(out=b_stage[:], in_=b[:, :])
            b_sb = b_pool.tile([P, d_out], mm_dt)
            nc.scalar.copy(b_sb[:], b_stage[:])

            n_copy = 0

            def evict(dst, src):
                nonlocal n_copy
                if n_copy % 2 == 0:
                    nc.vector.tensor_copy(dst, src)
                else:
                    nc.scalar.copy(dst, src)
                n_copy += 1

            itp_fixed = None
            if skip_tr and not skip_mm:
                itp_fixed = itp_pool.tile(
                    [P, n_kc, t_tile], mm_dt, tag="itp", name="itp_fixed"
                )
                nc.gpsimd.memset(itp_fixed[:].bitcast(F32), 0.5)
                # rounding no-op so the fp32r consumer passes BIR verification
                nc.vector.tensor_copy(itp_fixed[:], itp_fixed[:].bitcast(F32))

            for pss in range(passes):
                for tt in range(n_t_tiles):
                    if skip_mm:
                        # DMA round-trip probe: load rows, store them back out.
                        for j in range(n_j):
                            tb = tt * n_j + j
                            nat = nat_pool.tile([P, d_in], F32, tag="nat",
                                                name=f"nat{pss}_{tt}_{j}")
                            nc.sync.dma_start(
                                out=nat[:], in_=inp[tb * P : (tb + 1) * P, :]
                            )
                            nc.sync.dma_start(
                                out=outp[tb * P : (tb + 1) * P, :d_in],
                                in_=nat[:],
                            )
                        continue
                    if skip_tr:
                        itp = itp_fixed
                        for j in range(n_j):
                            tb = tt * n_j + j
                            nat = nat_pool.tile([P, d_in], F32, tag="nat",
                                                name=f"nat{pss}_{tt}_{j}")
                            nc.sync.dma_start(
                                out=nat[:], in_=inp[tb * P : (tb + 1) * P, :]
                            )
                    else:
                        itp = itp_pool.tile(
                            [P, n_kc, t_tile], mm_dt, tag="itp",
                            name=f"itp{pss}_{tt}",
                        )
                        for j in range(n_j):
                            tb = tt * n_j + j
                            nat = nat_pool.tile([P, d_in], F32, tag="nat",
                                                name=f"nat{pss}_{tt}_{j}")
                            nc.sync.dma_start(
                                out=nat[:], in_=inp[tb * P : (tb + 1) * P, :]
                            )
                            for q in range(n_q):
                                trp = tr_psum.tile([P, QUAD, P], F32, tag="trp",
                                                   name=f"trp{pss}_{tt}_{j}_{q}")
                                for i in range(QUAD):
                                    kc = q * QUAD + i
                                    nc.tensor.matmul(
                                        trp[:, i, :],
                                        nat[:, kc * P : (kc + 1) * P],
                                        identity[:],
                                        is_transpose=True,
                                        start=(i == 0),
                                        stop=(i == QUAD - 1),
                                    )
                                evict(
                                    itp[:, q * QUAD : (q + 1) * QUAD,
                                        j * P : (j + 1) * P],
                                    trp[:],
                                )
                    c1t_ps = c1t_psum.tile([P, t_tile], F32)
                    for kc in range(n_kc):
                        nc.tensor.matmul(
                            c1t_ps[:],
                            a_sb[:, kc * P : (kc + 1) * P],
                            itp[:, kc, :],
                            start=(kc == 0),
                            stop=(kc == n_kc - 1),
                        )
                    c1t = c1t_pool.tile([P, t_tile], mm_dt)
                    nc.vector.tensor_copy(c1t[:, : t_tile // 2], c1t_ps[:, : t_tile // 2])
                    nc.scalar.copy(c1t[:, t_tile // 2 :], c1t_ps[:, t_tile // 2 :])
                    for j in range(n_j):
                        tb = tt * n_j + j
                        for h in range(n_halves):
                            o_sb = out_pool.tile([P, out_cols], F32)
                            for qq in range(n_nc // n_halves):
                                ncol = h * (n_nc // n_halves) + qq
                                o_ps = out_psum.tile([P, FREE], F32)
                                nc.tensor.matmul(
                                    o_ps[:],
                                    c1t[:, j * P : (j + 1) * P],
                                    b_sb[:, ncol * FREE : (ncol + 1) * FREE],
                                    start=True,
                                    stop=True,
                                )
                                evict(o_sb[:, qq * FREE : (qq + 1) * FREE], o_ps[:])
                            (nc.scalar if store_act else nc.sync).dma_start(
                                out=outp[
                                    tb * P : (tb + 1) * P,
                                    h * out_cols : (h + 1) * out_cols,
                                ],
                                in_=o_sb[:],
                            )
    if legalize:
        _legalize_waits(nc)
    return nc


def build_nc3(
    t_core: int = T_FULL // N_CORES,
    d_in: int = D_IN,
    rank: int = RANK,
    d_out: int = D_OUT,
    mm_dt: mybir.dt = mybir.dt.float32r,
    legalize: bool = True,
    passes: int = 1,
    nat_bufs: int = 6,
    out_ps_bufs: int = 2,
    tr_ps_bufs: int = 4,
) -> bass.Bass:
    """v3 layout: quad-major transposes with mm1 interleaved right after each
    kc-quad completes (keeps matmuls flowing through the PE stream), per-quad
    itp tiles, deeper nat prefetch."""
    assert t_core % FREE == 0 and d_in % P == 0 and d_out % FREE == 0
    assert rank == P
    n_t_tiles = t_core // FREE
    n_j = FREE // P
    n_kc = d_in // P
    n_nc = d_out // FREE
    out_cols = min(d_out, 2048)
    n_halves = d_out // out_cols
    QUAD = 4
    n_q = n_kc // QUAD

    nc = bass.Bass()
    inp = nc.declare_dram_parameter("input", [t_core, d_in], F32, isOutput=False)
    a = nc.declare_dram_parameter("lora_A", [d_in, rank], F32, isOutput=False)
    b = nc.declare_dram_parameter("lora_B", [rank, d_out], F32, isOutput=False)
    outp = nc.declare_dram_parameter("output", [t_core, d_out], F32, isOutput=True)

    with TileContext(nc) as tc:
        with (
            tc.tile_pool(name="const", bufs=1) as const_pool,
            tc.tile_pool(name="a_sb", bufs=1) as a_pool,
            tc.tile_pool(name="b_sb", bufs=1) as b_pool,
            tc.tile_pool(name="nat", bufs=nat_bufs) as nat_pool,
            tc.tile_pool(name="itp", bufs=3) as itp_pool,
            tc.tile_pool(name="c1t_sb", bufs=2) as c1t_pool,
            tc.tile_pool(name="out_sb", bufs=2) as out_pool,
            tc.tile_pool(name="tr_ps", bufs=tr_ps_bufs, space="PSUM") as tr_psum,
            tc.tile_pool(name="c1t_ps", bufs=2, space="PSUM") as c1t_psum,
            tc.tile_pool(name="out_ps", bufs=out_ps_bufs, space="PSUM") as out_psum,
        ):
            identity = const_pool.tile([P, P], F32)
            make_identity(nc, identity)

            a_stage = nat_pool.tile([P, d_in], F32, tag="nat", name="a_stage")
            nc.sync.dma_start(
                out=a_stage[:].rearrange("p (kc r) -> p kc r", r=rank),
                in_=a.rearrange("(kc p) r -> p kc r", p=P),
            )
            a_sb = a_pool.tile([P, d_in], mm_dt)
            nc.vector.tensor_copy(a_sb[:], a_stage[:])
            b_stage = nat_pool.tile([P, d_out], F32, tag="nat", name="b_stage")
            nc.sync.dma_start(out=b_stage[:], in_=b[:, :])
            b_sb = b_pool.tile([P, d_out], mm_dt)
            nc.scalar.copy(b_sb[:], b_stage[:])

            n_copy = 0

            def evict(dst, src):
                nonlocal n_copy
                if n_copy % 2 == 0:
                    nc.vector.tensor_copy(dst, src)
                else:
                    nc.scalar.copy(dst, src)
                n_copy += 1

            for pss in range(passes):
                for tt in range(n_t_tiles):
                    nats = []
                    for j in range(n_j):
                        tb = tt * n_j + j
                        nat = nat_pool.tile([P, d_in], F32, tag="nat",
                                            name=f"nat{pss}_{tt}_{j}")
                        nc.sync.dma_start(
                            out=nat[:], in_=inp[tb * P : (tb + 1) * P, :]
                        )
                        nats.append(nat)
                    c1t_ps = c1t_psum.tile([P, FREE], F32)
                    for q in range(n_q):
                        itp = itp_pool.tile([P, QUAD, FREE], mm_dt, tag="itp",
                                            name=f"itp{pss}_{tt}_{q}")
                        for j in range(n_j):
                            trp = tr_psum.tile([P, QUAD, P], F32, tag="trp",
                                               name=f"trp{pss}_{tt}_{q}_{j}")
                            for i in range(QUAD):
                                kc = q * QUAD + i
                                nc.tensor.matmul(
                                    trp[:, i, :],
                                    nats[j][:, kc * P : (kc + 1) * P],
                                    identity[:],
                                    is_transpose=True,
                                    start=(i == 0),
                                    stop=(i == QUAD - 1),
                                )
                            evict(itp[:, :, j * P : (j + 1) * P], trp[:])
                        for i in range(QUAD):
                            kc = q * QUAD + i
                            nc.tensor.matmul(
                                c1t_ps[:],
                                a_sb[:, kc * P : (kc + 1) * P],
                                itp[:, i, :],
                                start=(kc == 0),
                                stop=(kc == n_kc - 1),
                            )
                    c1t = c1t_pool.tile([P, FREE], mm_dt)
                    nc.vector.tensor_copy(c1t[:, : FREE // 2], c1t_ps[:, : FREE // 2])
                    nc.scalar.copy(c1t[:, FREE // 2 :], c1t_ps[:, FREE // 2 :])
                    for j in range(n_j):
                        tb = tt * n_j + j
                        for h in range(n_halves):
                            o_sb = out_pool.tile([P, out_cols], F32)
                            for qq in range(n_nc // n_halves):
                                ncol = h * (n_nc // n_halves) + qq
                                o_ps = out_psum.tile([P, FREE], F32)
                                nc.tensor.matmul(
                                    o_ps[:],
                                    c1t[:, j * P : (j + 1) * P],
                                    b_sb[:, ncol * FREE : (ncol + 1) * FREE],
                                    start=True,
                                    stop=True,
                                )
                                evict(o_sb[:, qq * FREE : (qq + 1) * FREE], o_ps[:])
                            nc.sync.dma_start(
                                out=outp[
                                    tb * P : (tb + 1) * P,
                                    h * out_cols : (h + 1) * out_cols,
                                ],
                                in_=o_sb[:],
                            )
    if legalize:
        _legalize_waits(nc)
    return nc


def build_nc4(
    t_core: int = T_FULL // N_CORES,
    d_in: int = D_IN,
    rank: int = RANK,
    d_out: int = D_OUT,
    legalize: bool = True,
    passes: int = 1,
    t_tile: int = 512,  # tokens per tile (mm1 moving free dim)
    load_j: int = 2,  # 128-token blocks per input-load DMA
    tr_mode: str = "f32r2",  # 'f32r': f32r data x bf16 ident (1cyc/row) —
    #                           REJECTED by BIR verifier (no 32/16-bit mix);
    #                          'f32r2': f32r data x f32r ident (1.5cyc/row);
    #                          'f32': f32 data x f32 ident (2cyc/row);
    #                          'bf16': pre-cast input to bf16 (1cyc/row)
    itp_dt: mybir.dt = mybir.dt.bfloat16,  # transposed-input dtype (mm1 rhs)
    w_dt: mybir.dt = mybir.dt.bfloat16,  # A/B/c1t dtype (mm ops dtype)
    nat_bufs: int = 2,
    natb_bufs: int = 2,
    itp_bufs: int = 2,
    out_bufs: int = 2,
    wstage_bufs: int = 2,
    tr_ps_bufs: int = 4,
    c1t_ps_bufs: int = 2,
    out_ps_bufs: int = 2,
    evict_engines: str = "vs",  # subset of v(DVE) s(ACT), round-robin.
    #   (GPSIMD/Pool cannot read PSUM — it only gets SBUF->SBUF casts.)
    cast_engines: str = "p",  # engines splitting each input-block bf16 cast
    mm1_interleave: bool = False,  # q-outer transposes, mm1 per quad (nc3-style)
    store_sp: bool = False,  # issue output stores on the SP ring (not ACT)
    # --- timing probes (WRONG results; DMA traffic unchanged) ---
    probe_tr_quads: int | None = None,  # transpose only this many kc-quads/block
    probe_mm1: bool = True,  # False: skip mm1 (c1t memset once per tile)
    probe_nc: int | None = None,  # compute only this many out-cols per block
) -> bass.Bass:
    """v4: bf16 matmul pipeline.

    Input blocks are DMAed as f32, cast to bf16 on the Pool engine
    (SBUF->SBUF; Pool cannot touch PSUM), transposed on the PE at
    1 cycle/row (bf16 data x bf16 identity), and evicted from PSUM by
    DVE/ACT round-robin into a bf16 itp tile. mm1 (A.T @ inputT) and
    mm2 (C1.T @ B) run entirely in bf16 (PSUM accumulation stays f32),
    which the 2e-2 tolerance easily absorbs (measured ~1.9e-3)."""
    assert t_core % t_tile == 0 and d_in % P == 0 and d_out % FREE == 0
    assert rank == P and t_tile >= 256
    n_t_tiles = t_core // t_tile
    n_j = t_tile // P
    assert n_j % load_j == 0
    n_kc = d_in // P
    n_nc = d_out // FREE
    QUAD = 4
    n_q = n_kc // QUAD
    BF16 = mybir.dt.bfloat16
    F32R = mybir.dt.float32r

    nat_dt = {"f32r": F32R, "f32r2": F32R, "f32": F32, "bf16": F32}[tr_mode]
    ident_dt = {"f32r": BF16, "f32r2": F32R, "f32": F32, "bf16": BF16}[tr_mode]
    trp_dt = {"f32r": F32R, "f32r2": F32R, "f32": F32, "bf16": BF16}[tr_mode]

    nc = bass.Bass()
    inp = nc.declare_dram_parameter("input", [t_core, d_in], F32, isOutput=False)
    a = nc.declare_dram_parameter("lora_A", [d_in, rank], F32, isOutput=False)
    b = nc.declare_dram_parameter("lora_B", [rank, d_out], F32, isOutput=False)
    outp = nc.declare_dram_parameter("output", [t_core, d_out], F32, isOutput=True)

    with TileContext(nc) as tc:
        with (
            tc.tile_pool(name="const", bufs=1) as const_pool,
            tc.tile_pool(name="a_sb", bufs=1) as a_pool,
            tc.tile_pool(name="b_sb", bufs=1) as b_pool,
            tc.tile_pool(name="nat", bufs=nat_bufs) as nat_pool,
            tc.tile_pool(name="natb", bufs=natb_bufs) as natb_pool,
            tc.tile_pool(name="itp", bufs=itp_bufs) as itp_pool,
            tc.tile_pool(name="c1t_sb", bufs=2) as c1t_pool,
            tc.tile_pool(name="out_sb", bufs=out_bufs) as out_pool,
            tc.tile_pool(name="wstage", bufs=wstage_bufs) as wstage_pool,
            tc.tile_pool(name="tr_ps", bufs=tr_ps_bufs, space="PSUM") as tr_psum,
            tc.tile_pool(name="c1t_ps", bufs=c1t_ps_bufs, space="PSUM") as c1t_psum,
            tc.tile_pool(name="out_ps", bufs=out_ps_bufs, space="PSUM") as out_psum,
        ):
            identity = const_pool.tile([P, P], ident_dt)
            make_identity(nc, identity)

            # Weights: stage f32 via DMA, cast-copy into matmul dtype.
            # Own pool — sharing the nat pool would serialize the first
            # input loads behind the weight staging buffers.
            a_stage = wstage_pool.tile([P, d_in], F32, tag="w", name="a_stage")
            nc.sync.dma_start(
                out=a_stage[:].rearrange("p (kc r) -> p kc r", r=rank),
                in_=a.rearrange("(kc p) r -> p kc r", p=P),
            )
            a_sb = a_pool.tile([P, n_kc, rank], w_dt)
            nc.vector.tensor_copy(
                a_sb[:], a_stage[:].rearrange("p (kc r) -> p kc r", r=rank)
            )
            b_stage = wstage_pool.tile([P, d_out], F32, tag="w", name="b_stage")
            nc.sync.dma_start(out=b_stage[:], in_=b[:, :])
            b_sb = b_pool.tile([P, d_out], w_dt)
            nc.scalar.copy(b_sb[:], b_stage[:])

            engines = {"v": nc.vector, "s": nc.scalar, "p": nc.gpsimd}
            ring = [engines[ch] for ch in evict_engines]
            n_copy = 0

            def evict(dst, src):
                nonlocal n_copy
                eng = ring[n_copy % len(ring)]
                if eng is nc.scalar:
                    eng.copy(dst, src)
                else:
                    eng.tensor_copy(dst, src)
                n_copy += 1

            for pss in range(passes):
                for tt in range(n_t_tiles):
                    if mm1_interleave:
                        # q-outer ordering: all of this tile's input blocks are
                        # cast first; each kc-quad is transposed for every j
                        # and immediately consumed by mm1, so mm1 progresses
                        # inside the transpose phase (shorter fill/drain).
                        assert tr_mode == "bf16" and load_j == 1
                        assert probe_tr_quads is None and probe_mm1 and probe_nc is None
                        n_ce = len(cast_engines)
                        bounds = [
                            (d_in * ci // n_ce) // P * P for ci in range(n_ce)
                        ] + [d_in]
                        natbs = []
                        for j in range(n_j):
                            nat = nat_pool.tile(
                                [P, 1, d_in], F32, tag="nat",
                                name=f"nat{pss}_{tt}_{j}",
                            )
                            t0 = tt * t_tile + j * P
                            nc.sync.dma_start(
                                out=nat[:],
                                in_=inp[t0 : t0 + P, :].rearrange(
                                    "(j p) k -> p j k", p=P
                                ),
                            )
                            natb = natb_pool.tile(
                                [P, d_in], BF16, tag="natb",
                                name=f"natb{pss}_{tt}_{j}",
                            )
                            for ci, ch in enumerate(cast_engines):
                                lo, hi = bounds[ci], bounds[ci + 1]
                                eng = engines[ch]
                                if eng is nc.scalar:
                                    eng.copy(natb[:, lo:hi], nat[:, 0, lo:hi])
                                else:
                                    eng.tensor_copy(natb[:, lo:hi], nat[:, 0, lo:hi])
                            natbs.append(natb)
                        c1t_ps = c1t_psum.tile([P, t_tile], F32)
                        for q in range(n_q):
                            itp_q = itp_pool.tile(
                                [P, QUAD, t_tile], itp_dt, tag="itp",
                                name=f"itpq{pss}_{tt}_{q}",
                            )
                            for j in range(n_j):
                                trp = tr_psum.tile(
                                    [P, QUAD, P], trp_dt, tag="trp",
                                    name=f"trp{pss}_{tt}_{q}_{j}",
                                )
                                for i in range(QUAD):
                                    kc = q * QUAD + i
                                    nc.tensor.matmul(
                                        trp[:, i, :],
                                        natbs[j][:, kc * P : (kc + 1) * P],
                                        identity[:],
                                        is_transpose=True,
                                        start=(i == 0),
                                        stop=(i == QUAD - 1),
                                    )
                                evict(
                                    itp_q[:, :, j * P : (j + 1) * P], trp[:]
                                )
                            for i in range(QUAD):
                                kc = q * QUAD + i
                                nc.tensor.matmul(
                                    c1t_ps[:],
                                    a_sb[:, kc, :],
                                    itp_q[:, i, :],
                                    start=(kc == 0),
                                    stop=(kc == n_kc - 1),
                                )
                        c1t = c1t_pool.tile([P, t_tile], w_dt)
                        half = t_tile // 2
                        nc.vector.tensor_copy(c1t[:, :half], c1t_ps[:, :half])
                        nc.scalar.copy(c1t[:, half:], c1t_ps[:, half:])
                        for j in range(n_j):
                            tb = tt * n_j + j
                            o_sb = out_pool.tile([P, d_out], F32)
                            for q in range(n_nc):
                                o_ps = out_psum.tile([P, FREE], F32)
                                nc.tensor.matmul(
                                    o_ps[:],
                                    c1t[:, j * P : (j + 1) * P],
                                    b_sb[:, q * FREE : (q + 1) * FREE],
                                    start=True,
                                    stop=True,
                                )
                                evict(o_sb[:, q * FREE : (q + 1) * FREE], o_ps[:])
                            (nc.sync if store_sp else nc.scalar).dma_start(
                                out=outp[tb * P : (tb + 1) * P, :],
                                in_=o_sb[:],
                            )
                        continue
                    itp = itp_pool.tile(
                        [P, n_kc, t_tile], itp_dt, tag="itp",
                        name=f"itp{pss}_{tt}",
                    )
                    for jj in range(n_j // load_j):
                        nat = nat_pool.tile(
                            [P, load_j, d_in], nat_dt, tag="nat",
                            name=f"nat{pss}_{tt}_{jj}",
                        )
                        t0 = tt * t_tile + jj * load_j * P
                        nc.sync.dma_start(
                            out=nat[:].bitcast(F32) if nat_dt == F32R else nat[:],
                            in_=inp[t0 : t0 + load_j * P, :].rearrange(
                                "(j p) k -> p j k", p=P
                            ),
                        )
                        if tr_mode == "bf16":
                            natb = natb_pool.tile(
                                [P, load_j, d_in], BF16, tag="natb",
                                name=f"natb{pss}_{tt}_{jj}",
                            )
                            n_ce = len(cast_engines)
                            bounds = [
                                (d_in * ci // n_ce) // P * P for ci in range(n_ce)
                            ] + [d_in]
                            for j2 in range(load_j):
                                for ci, ch in enumerate(cast_engines):
                                    lo, hi = bounds[ci], bounds[ci + 1]
                                    eng = engines[ch]
                                    if eng is nc.scalar:
                                        eng.copy(
                                            natb[:, j2, lo:hi], nat[:, j2, lo:hi]
                                        )
                                    else:
                                        eng.tensor_copy(
                                            natb[:, j2, lo:hi], nat[:, j2, lo:hi]
                                        )
                            src_t = natb
                        else:
                            src_t = nat
                        for j2 in range(load_j):
                            j = jj * load_j + j2
                            for q in range(n_q if probe_tr_quads is None
                                           else probe_tr_quads):
                                trp = tr_psum.tile(
                                    [P, QUAD, P], trp_dt, tag="trp",
                                    name=f"trp{pss}_{tt}_{j}_{q}",
                                )
                                for i in range(QUAD):
                                    kc = q * QUAD + i
                                    nc.tensor.matmul(
                                        trp[:, i, :],
                                        src_t[:, j2, kc * P : (kc + 1) * P],
                                        identity[:],
                                        is_transpose=True,
                                        start=(i == 0),
                                        stop=(i == QUAD - 1),
                                    )
                                evict(
                                    itp[:, q * QUAD : (q + 1) * QUAD,
                                        j * P : (j + 1) * P],
                                    trp[:],
                                )
                    kc_mm1 = (
                        range(n_kc)
                        if probe_tr_quads is None
                        else range(probe_tr_quads * QUAD)
                    )
                    c1t = c1t_pool.tile([P, t_tile], w_dt)
                    if probe_mm1:
                        c1t_ps = c1t_psum.tile([P, t_tile], F32)
                        for kc in kc_mm1:
                            nc.tensor.matmul(
                                c1t_ps[:],
                                a_sb[:, kc, :],
                                itp[:, kc, :],
                                start=(kc == kc_mm1[0]),
                                stop=(kc == kc_mm1[-1]),
                            )
                        half = t_tile // 2
                        nc.vector.tensor_copy(c1t[:, :half], c1t_ps[:, :half])
                        nc.scalar.copy(c1t[:, half:], c1t_ps[:, half:])
                    else:
                        nc.gpsimd.memset(c1t[:], 0.25)
                    for j in range(n_j):
                        tb = tt * n_j + j
                        o_sb = out_pool.tile([P, d_out], F32)
                        for q in range(n_nc if probe_nc is None else probe_nc):
                            o_ps = out_psum.tile([P, FREE], F32)
                            nc.tensor.matmul(
                                o_ps[:],
                                c1t[:, j * P : (j + 1) * P],
                                b_sb[:, q * FREE : (q + 1) * FREE],
                                start=True,
                                stop=True,
                            )
                            evict(o_sb[:, q * FREE : (q + 1) * FREE], o_ps[:])
                        nc.scalar.dma_start(
                            out=outp[tb * P : (tb + 1) * P, :],
                            in_=o_sb[:],
                        )
    if legalize:
        _legalize_waits(nc)
    return nc


_NC_CACHE: dict[tuple, bass.Bass] = {}


# Best measured config: v4 layout — bf16 matmul pipeline (input cast split
# Pool+DVE, 1-cycle/row bf16 PE transposes, bf16 mm1/mm2 with a 512-wide
# mm1 moving free dim), single-block loads with nat prefetch, full-row
# output stores on the ACT ring, and q-outer transpose ordering with mm1
# consuming each kc-quad as soon as it lands (mm1_interleave).
BEST_KW = dict(
    t_tile=512, load_j=1, nat_bufs=3, natb_bufs=5, itp_bufs=3,
    cast_engines="pv", tr_mode="bf16", mm1_interleave=True,
    tr_ps_bufs=3, c1t_ps_bufs=1, out_ps_bufs=4,
)


def _get_nc(**kw) -> bass.Bass:
    kw = {**BEST_KW, **kw}
    key = tuple(sorted(kw.items()))
    if key not in _NC_CACHE:
        _NC_CACHE[key] = build_nc4(**kw)
    return _NC_CACHE[key]


def kernel(input: np.ndarray, lora_A: np.ndarray, lora_B: np.ndarray) -> np.ndarray:
    input = np.ascontiguousarray(np.asarray(input, dtype=np.float32))
    lora_A = np.ascontiguousarray(np.asarray(lora_A, dtype=np.float32))
    lora_B = np.ascontiguousarray(np.asarray(lora_B, dtype=np.float32))
    assert input.shape == (T_FULL, D_IN), input.shape
    assert lora_A.shape == (D_IN, RANK), lora_A.shape
    assert lora_B.shape == (RANK, D_OUT), lora_B.shape

    t_core = T_FULL // N_CORES
    shards = input.reshape(N_CORES, t_core, D_IN)
    nc = _get_nc()
    in_maps = [
        {"input": shards[i], "lora_A": lora_A, "lora_B": lora_B}
        for i in range(N_CORES)
    ]
    res = run_bass_kernel_spmd(nc, in_maps, list(range(N_CORES)))
    return np.concatenate(
        [res.results[i]["output"] for i in range(N_CORES)], axis=0
    )

